# revision 9
# baseline (speedup 1.0000x reference)
"""Greedy CTC decoder on Trainium2 (Bass/Tile), sharded over 8 NeuronCores.

Input : emission [65536, 512] float32 (full, unsharded)
Output: (index [65536] int32, keep [65536] bool) matching the reference:
    index = argmax(emission, axis=-1)
    char  = index - 1 (blank 0 -> -1)
    keep  = (char != prev_char) & (char != -1)
          = (index != prev_index) & (index != 0),  prev of t=0 is a sentinel

Sharding: timestep axis T split across 8 cores (8192 rows each). Inside a
core, partition p owns the 64 consecutive timesteps p*64..p*64+63, so the
repeat-collapse comparison is a free-dim shift. The 64-step chunk boundary
(prev of j=0 lives on partition p-1) is resolved with one tiny SBUF->SBUF
DMA; the 7 shard boundaries are fixed on the host.
"""

import numpy as np

import concourse.bacc as bacc
import concourse.mybir as mybir
from concourse.tile import TileContext
from concourse.bass_utils import run_bass_kernel_spmd

N_CORES = 8
T_FULL = 65536
V = 512
P = 128
T_SHARD = T_FULL // N_CORES          # 8192
JPP = T_SHARD // P                   # 64 timesteps per partition
# chunk sizes (timesteps per partition per DMA): small first chunks so the
# DVE starts early, 2 MiB chunks later for full DMA efficiency
CHUNKS = [2, 2, 4] + [8] * 7
HALF = 32                            # keep-mask split point (after 6 chunks)
SENTINEL = 1000000.0                 # != any vocab index, exact in fp32

_prog_cache = {}


def _build():
    nc = bacc.Bacc(None, target_bir_lowering=False)

    em_h = nc.dram_tensor("emission", [T_SHARD, V], mybir.dt.float32,
                          kind="ExternalInput")
    idx_h = nc.dram_tensor("idx_out", [T_SHARD], mybir.dt.uint32,
                           kind="ExternalOutput")
    keep_h = nc.dram_tensor("keep_out", [T_SHARD], mybir.dt.uint8,
                            kind="ExternalOutput")

    # [T_SHARD, V] -> [P, JPP, V]: partition p holds rows p*JPP .. p*JPP+JPP-1
    em3 = em_h[:, :].rearrange("(p j) v -> p j v", p=P)
    idx_out2 = idx_h[:].rearrange("(p j) -> p j", p=P)
    keep_out2 = keep_h[:].rearrange("(p j) -> p j", p=P)

    with TileContext(nc) as tc:
        with (
            tc.tile_pool(name="io", bufs=4) as io_pool,
            tc.tile_pool(name="mx", bufs=4) as mx_pool,
            tc.tile_pool(name="acc", bufs=1) as acc_pool,
        ):
            # argmax ids for all 64 rows per partition, 8 slots per row
            # (max_index writes 8 indices; slot 0 is the argmax)
            idx8 = acc_pool.tile([P, JPP, 8], mybir.dt.uint32)
            idxc = acc_pool.tile([P, JPP], mybir.dt.uint32)
            neq = acc_pool.tile([P, JPP], mybir.dt.uint8)
            nz = acc_pool.tile([P, JPP], mybir.dt.uint8)
            keep = acc_pool.tile([P, JPP], mybir.dt.uint8)
            prev0 = acc_pool.tile([P, 1], mybir.dt.uint32)

            def keep_phase(lo, hi):
                """Repeat-collapse for columns [lo, hi) on GpSimd (DVE stays
                on max_index). Column 0 is deferred to the caller."""
                v = nc.vector
                # compact argmax (slot 0 of each 8-group) to contiguous u32
                v.tensor_copy(idxc[:, lo:hi], idx8[:, lo:hi, 0])
                lo1 = max(lo, 1)  # column 0 needs the cross-partition prev
                v.tensor_tensor(out=neq[:, lo1:hi], in0=idxc[:, lo1:hi],
                                in1=idxc[:, lo1 - 1:hi - 1],
                                op=mybir.AluOpType.not_equal)
                v.tensor_scalar(out=nz[:, lo:hi], in0=idxc[:, lo:hi],
                                scalar1=0.0, scalar2=None,
                                op0=mybir.AluOpType.not_equal)
                v.tensor_tensor(out=keep[:, lo1:hi], in0=neq[:, lo1:hi],
                                in1=nz[:, lo1:hi], op=mybir.AluOpType.mult)
                nc.sync.dma_start(out=idx_out2[:, lo:hi], in_=idxc[:, lo:hi])
                nc.sync.dma_start(out=keep_out2[:, lo1:hi],
                                  in_=keep[:, lo1:hi])

            j = 0
            for c, n in enumerate(CHUNKS):
                tile = io_pool.tile([P, n, V], mybir.dt.float32)
                nc.sync.dma_start(out=tile[:, :, :], in_=em3[:, j:j + n, :])
                # one reduce for all n rows' maxes (552ns/row vs 608 for
                # per-row InstMax)
                rowmax = mx_pool.tile([P, n], mybir.dt.float32)
                nc.vector.tensor_reduce(out=rowmax[:, :], in_=tile[:, :, :],
                                        axis=mybir.AxisListType.X,
                                        op=mybir.AluOpType.max)
                for k in range(n):
                    nc.vector.max_index(
                        out=idx8[:, j + k, :],
                        in_max=rowmax[:, k:k + 1].broadcast_to((P, 8)),
                        in_values=tile[:, k, :])
                j += n
                if j == HALF:
                    keep_phase(0, HALF)

            keep_phase(HALF, JPP)

            # column 0: prev = last timestep of the previous partition;
            # partition 0 gets the sentinel (fixed on host for shards 1..7,
            # genuinely "no prev" for shard 0)
            nc.vector.memset(prev0[:, :], int(SENTINEL))
            nc.sync.dma_start(out=prev0[1:P, :], in_=idxc[0:P - 1, JPP - 1:JPP])
            nc.vector.tensor_tensor(out=neq[:, 0:1], in0=idxc[:, 0:1],
                                    in1=prev0[:, :],
                                    op=mybir.AluOpType.not_equal)
            nc.vector.tensor_tensor(out=keep[:, 0:1], in0=neq[:, 0:1],
                                    in1=nz[:, 0:1], op=mybir.AluOpType.mult)
            nc.sync.dma_start(out=keep_out2[:, 0:1], in_=keep[:, 0:1])

    nc.compile()
    return nc


def _get_prog():
    if "nc" not in _prog_cache:
        _prog_cache["nc"] = _build()
    return _prog_cache["nc"]


def run_sharded(emission: np.ndarray, **spmd_kwargs):
    """Run the SPMD kernel; returns (idx int32 [T], keep bool [T], results)."""
    emission = np.ascontiguousarray(np.asarray(emission, dtype=np.float32))
    assert emission.shape == (T_FULL, V), emission.shape
    nc = _get_prog()
    in_maps = [
        {"emission": np.ascontiguousarray(emission[c * T_SHARD:(c + 1) * T_SHARD])}
        for c in range(N_CORES)
    ]
    res = run_bass_kernel_spmd(nc, in_maps, list(range(N_CORES)), **spmd_kwargs)
    idx = np.concatenate([res.results[c]["idx_out"] for c in range(N_CORES)])
    keep = np.concatenate([res.results[c]["keep_out"] for c in range(N_CORES)])
    idx = idx.astype(np.int32, copy=False)
    keep = keep.astype(bool, copy=False)
    # boundary exchange: fix the first timestep of shards 1..7
    for c in range(1, N_CORES):
        t = c * T_SHARD
        keep[t] = (idx[t] != idx[t - 1]) and (idx[t] != 0)
    return idx, keep, res


def kernel(emission: np.ndarray):
    idx, keep, _ = run_sharded(emission)
    return idx, keep


# revision 10
# speedup vs baseline: 1.0538x; 1.0538x over previous
"""Greedy CTC decoder on Trainium2 (Bass/Tile), sharded over 8 NeuronCores.

Input : emission [65536, 512] float32 (full, unsharded)
Output: (index [65536] int32, keep [65536] bool) matching the reference:
    index = argmax(emission, axis=-1)
    char  = index - 1 (blank 0 -> -1)
    keep  = (char != prev_char) & (char != -1)
          = (index != prev_index) & (index != 0),  prev of t=0 is a sentinel

Sharding: timestep axis T split across 8 cores (8192 rows each). Inside a
core, partition p owns the 64 consecutive timesteps p*64..p*64+63, so the
repeat-collapse comparison is a free-dim shift. The 64-step chunk boundary
(prev of j=0 lives on partition p-1) is resolved with one tiny SBUF->SBUF
DMA; the 7 shard boundaries are fixed on the host.
"""

import numpy as np

import concourse.bacc as bacc
import concourse.mybir as mybir
from concourse.tile import TileContext
from concourse.bass_utils import run_bass_kernel_spmd

N_CORES = 8
T_FULL = 65536
V = 512
P = 128
T_SHARD = T_FULL // N_CORES          # 8192
JPP = T_SHARD // P                   # 64 timesteps per partition
# chunk sizes (timesteps per partition per DMA): small first chunks so the
# DVE starts early, 2 MiB chunks later for full DMA efficiency
CHUNKS = [2, 2, 4] + [8] * 7
HALF = 32                            # keep-mask split point (after 6 chunks)
SENTINEL = 1000000.0                 # != any vocab index, exact in fp32

_prog_cache = {}


def _build():
    nc = bacc.Bacc(None, target_bir_lowering=False)

    em_h = nc.dram_tensor("emission", [T_SHARD, V], mybir.dt.float32,
                          kind="ExternalInput")
    idx_h = nc.dram_tensor("idx_out", [T_SHARD], mybir.dt.uint32,
                           kind="ExternalOutput")
    keep_h = nc.dram_tensor("keep_out", [T_SHARD], mybir.dt.uint8,
                            kind="ExternalOutput")

    # [T_SHARD, V] -> [P, JPP, V]: partition p holds rows p*JPP .. p*JPP+JPP-1
    em3 = em_h[:, :].rearrange("(p j) v -> p j v", p=P)
    idx_out2 = idx_h[:].rearrange("(p j) -> p j", p=P)
    keep_out2 = keep_h[:].rearrange("(p j) -> p j", p=P)

    with TileContext(nc) as tc:
        with (
            tc.tile_pool(name="io", bufs=4) as io_pool,
            tc.tile_pool(name="mx", bufs=4) as mx_pool,
            tc.tile_pool(name="acc", bufs=1) as acc_pool,
        ):
            # argmax ids for all 64 rows per partition, 8 slots per row
            # (max_index writes 8 indices; slot 0 is the argmax)
            idx8 = acc_pool.tile([P, JPP, 8], mybir.dt.uint32)
            idxc = acc_pool.tile([P, JPP], mybir.dt.uint32)
            neq = acc_pool.tile([P, JPP], mybir.dt.uint8)
            nz = acc_pool.tile([P, JPP], mybir.dt.uint8)
            keep = acc_pool.tile([P, JPP], mybir.dt.uint8)

            def keep_phase(lo, hi):
                """Repeat-collapse for columns [lo, hi) on GpSimd (DVE stays
                on max_index). Column 0 is deferred to the caller."""
                v = nc.vector
                # compact argmax (slot 0 of each 8-group) to contiguous u32
                v.tensor_copy(idxc[:, lo:hi], idx8[:, lo:hi, 0])
                lo1 = max(lo, 1)  # column 0 needs the cross-partition prev
                v.tensor_tensor(out=neq[:, lo1:hi], in0=idxc[:, lo1:hi],
                                in1=idxc[:, lo1 - 1:hi - 1],
                                op=mybir.AluOpType.not_equal)
                v.tensor_scalar(out=nz[:, lo:hi], in0=idxc[:, lo:hi],
                                scalar1=0.0, scalar2=None,
                                op0=mybir.AluOpType.not_equal)
                v.tensor_tensor(out=keep[:, lo1:hi], in0=neq[:, lo1:hi],
                                in1=nz[:, lo1:hi], op=mybir.AluOpType.mult)
                nc.sync.dma_start(out=idx_out2[:, lo:hi], in_=idxc[:, lo:hi])
                nc.sync.dma_start(out=keep_out2[:, lo1:hi],
                                  in_=keep[:, lo1:hi])

            j = 0
            for c, n in enumerate(CHUNKS):
                tile = io_pool.tile([P, n, V], mybir.dt.float32)
                nc.sync.dma_start(out=tile[:, :, :], in_=em3[:, j:j + n, :])
                # one reduce for all n rows' maxes (552ns/row vs 608 for
                # per-row InstMax)
                rowmax = mx_pool.tile([P, n], mybir.dt.float32)
                nc.vector.tensor_reduce(out=rowmax[:, :], in_=tile[:, :, :],
                                        axis=mybir.AxisListType.X,
                                        op=mybir.AluOpType.max)
                for k in range(n):
                    nc.vector.max_index(
                        out=idx8[:, j + k, :],
                        in_max=rowmax[:, k:k + 1].broadcast_to((P, 8)),
                        in_values=tile[:, k, :])
                j += n
                if j == HALF:
                    keep_phase(0, HALF)

            keep_phase(HALF, JPP)
            # column 0 of each partition (t % 64 == 0) is resolved on the
            # host: it needs the previous partition/shard's last index, and
            # a 128-byte cross-partition DMA costs ~3us of tail latency here

    nc.compile()
    return nc


def _get_prog():
    if "nc" not in _prog_cache:
        _prog_cache["nc"] = _build()
    return _prog_cache["nc"]


def run_sharded(emission: np.ndarray, **spmd_kwargs):
    """Run the SPMD kernel; returns (idx int32 [T], keep bool [T], results)."""
    emission = np.ascontiguousarray(np.asarray(emission, dtype=np.float32))
    assert emission.shape == (T_FULL, V), emission.shape
    nc = _get_prog()
    in_maps = [
        {"emission": np.ascontiguousarray(emission[c * T_SHARD:(c + 1) * T_SHARD])}
        for c in range(N_CORES)
    ]
    res = run_bass_kernel_spmd(nc, in_maps, list(range(N_CORES)), **spmd_kwargs)
    idx = np.concatenate([res.results[c]["idx_out"] for c in range(N_CORES)])
    keep = np.concatenate([res.results[c]["keep_out"] for c in range(N_CORES)])
    idx = idx.astype(np.int32, copy=False)
    keep = keep.astype(bool, copy=False)
    # boundary exchange: the device leaves every 64-step chunk's first
    # timestep unresolved (cross-partition/shard prev); fix them all here
    b = np.arange(64, T_FULL, 64)
    keep[b] = (idx[b] != idx[b - 1]) & (idx[b] != 0)
    keep[0] = idx[0] != 0
    return idx, keep, res


def kernel(emission: np.ndarray):
    idx, keep, _ = run_sharded(emission)
    return idx, keep
